# revision 1
# baseline (speedup 1.0000x reference)
"""BitLinear158 forward on 8 Trainium2 NeuronCores.

y = x @ quantize(W).T where quantize is the absmean ternary quantizer:
    gamma = mean(|W|) + 1e-6 ; qw = sign(W) * min(round(|W/gamma|), 1)

Strategy (tensor parallel over out_features, x replicated):
  - host: compute the exact fp32 threshold T such that qw != 0 <=> |w| > T
    (T is derived from a bit-exact replication of the reference quantizer;
    the division-based quantizer is monotone in |w|, so a single scalar
    threshold reproduces it exactly on the given weight).
  - each core: DMA its W.T shard, quantize on-device to ternary bf16
    ({-1,0,1} are exact in bf16), then compute x @ qw.T with the PE array
    using a dual-bf16 split of x (x = hi + lo) accumulated in fp32 PSUM,
    which gives ~fp32 accuracy at 2 bf16 passes.
  - layouts are pre-transposed on host (xT, wT) so every DMA is a clean
    partition-major access pattern and no on-chip transposes are needed.
"""

import numpy as np

import concourse.bass as bass
import concourse.bacc as bacc
import concourse.mybir as mybir
import concourse.tile as tile
from concourse import bass_utils

# Problem shapes (hardcoded per contract).
B, S, D_IN, D_OUT = 4, 2048, 4096, 16384
N_CORES = 8
O_PER = D_OUT // N_CORES          # 2048 out-features per core
T_TOK = B * S                     # 8192 tokens
EPS = 1e-6

# Set by test harness to capture profiling info; leave False for grading.
TRACE = False
TMPDIR = None
LAST_RESULTS = None


def _quantize_ref(weight: np.ndarray) -> np.ndarray:
    """Bit-exact replication of reference.absmean_quantize (eager jax on the
    default backend, matching how the reference executes); numpy fallback."""
    try:
        import jax.numpy as jnp

        gamma = jnp.abs(weight).mean() + EPS
        ws = weight / gamma
        qw = jnp.sign(ws) * jnp.minimum(jnp.round(jnp.abs(ws)), 1.0)
        return np.asarray(qw)
    except Exception:
        gamma = np.float32(np.abs(weight).mean(dtype=np.float64)) + np.float32(EPS)
        ws = (weight / gamma).astype(np.float32)
        return (np.sign(ws) * np.minimum(np.round(np.abs(ws)), np.float32(1.0))
                ).astype(np.float32)


def _threshold(weight: np.ndarray) -> float:
    """Largest |w| that quantizes to 0. Then (|w| > T) <=> (qw != 0),
    exactly, by monotonicity of the quantizer in |w|."""
    qw = _quantize_ref(weight)
    aw = np.abs(weight)
    zeros = qw == 0
    if zeros.any():
        t = np.float32(aw[zeros].max())
    else:
        t = np.float32(0.0)
    mism = int(((aw > t) != (qw != 0)).sum())
    if mism:
        # should be impossible (monotonicity); threshold is still the best
        # separator, so proceed.
        print(f"kernel.py: threshold calibration mismatches: {mism}")
    return float(t)


def build_program(thresh: float, t_tiles: int = T_TOK // 128,
                  o_per: int = O_PER, ks: int = D_IN // 128) -> bass.Bass:
    """Emit the per-core Bass/Tile program.

    DRAM I/O (per core):
      xT [ks*128, t_tiles*128] f32  -- x transposed, replicated
      wT [ks*128, o_per]       f32  -- this core's W.T shard
      y  [t_tiles*128, o_per]  f32  -- this core's output slice
    """
    K = ks * 128
    T = t_tiles * 128
    NCHUNK = o_per // 512
    XH = 2                       # x staged in halves of ks//XH slabs
    HS = ks // XH

    nc = bacc.Bacc("TRN2", target_bir_lowering=False, debug=False)
    xT = nc.dram_tensor("xT", [K, T], mybir.dt.float32, kind="ExternalInput")
    wT = nc.dram_tensor("wT", [K, o_per], mybir.dt.float32,
                        kind="ExternalInput")
    y = nc.dram_tensor("y", [T, o_per], mybir.dt.float32,
                       kind="ExternalOutput")

    xTr = xT.ap().rearrange("(k p) t -> p k t", p=128)
    wTr = wT.ap().rearrange("(k p) o -> p k o", p=128)

    with tile.TileContext(nc) as tc:
        with (
            tc.tile_pool(name="qw", bufs=1) as qw_pool,
            tc.tile_pool(name="wstage", bufs=2) as wstage_pool,
            tc.tile_pool(name="xstage", bufs=1) as xstage_pool,
            tc.tile_pool(name="qtmp", bufs=1) as qtmp_pool,
            tc.tile_pool(name="xhi", bufs=2) as xhi_pool,
            tc.tile_pool(name="xlo", bufs=2) as xlo_pool,
            tc.tile_pool(name="outs", bufs=2) as out_pool,
            tc.tile_pool(name="psum", bufs=2, space="PSUM") as psum_pool,
        ):
            def x_convert(t):
                """Load+split one 128-token tile of xT into bf16 hi/lo."""
                xhi = xhi_pool.tile([128, ks, 128], mybir.dt.bfloat16,
                                    name="xhi", tag="xhi")
                xlo = xlo_pool.tile([128, ks, 128], mybir.dt.bfloat16,
                                    name="xlo", tag="xlo")
                for hf in range(XH):
                    xst = xstage_pool.tile([128, HS, 128], mybir.dt.float32,
                                           name="xst", tag="xst")
                    nc.gpsimd.dma_start(
                        out=xst,
                        in_=xTr[:, hf * HS:(hf + 1) * HS,
                                t * 128:(t + 1) * 128],
                    )
                    hs = slice(hf * HS, (hf + 1) * HS)
                    nc.vector.tensor_copy(out=xhi[:, hs, :], in_=xst)
                    # mixed-dtype subtract: f32 - bf16 -> bf16
                    nc.vector.tensor_tensor(
                        out=xlo[:, hs, :], in0=xst, in1=xhi[:, hs, :],
                        op=mybir.AluOpType.subtract,
                    )
                return xhi, xlo

            # First token tile's x conversion is emitted ahead of the
            # quantize loop so the PE can start as soon as slab 0 lands.
            xcur = x_convert(0)

            # ---- quantize weight shard to ternary bf16, kept resident ----
            qw = qw_pool.tile([128, ks, o_per], mybir.dt.bfloat16)
            for k in range(ks):
                wst = wstage_pool.tile([128, o_per], mybir.dt.float32)
                nc.gpsimd.dma_start(out=wst, in_=wTr[:, k, :])
                lt = qtmp_pool.tile([128, o_per], mybir.dt.bfloat16)
                nc.vector.tensor_scalar(
                    out=qw[:, k, :], in0=wst, scalar1=thresh, scalar2=None,
                    op0=mybir.AluOpType.is_gt,
                )
                nc.vector.tensor_scalar(
                    out=lt, in0=wst, scalar1=-thresh, scalar2=None,
                    op0=mybir.AluOpType.is_lt,
                )
                nc.vector.tensor_tensor(
                    out=qw[:, k, :], in0=qw[:, k, :], in1=lt,
                    op=mybir.AluOpType.subtract,
                )

            # ---- main loop over 128-token tiles ----
            for t in range(t_tiles):
                xhi, xlo = xcur
                if t + 1 < t_tiles:
                    xnext = x_convert(t + 1)

                ot = out_pool.tile([128, o_per], mybir.dt.float32)
                pss = [psum_pool.tile([128, 512], mybir.dt.float32,
                                      name=f"ps{c}", tag=f"ps{c}")
                       for c in range(NCHUNK)]
                for k in range(ks):
                    for h, xb in ((0, xhi), (1, xlo)):
                        for c in range(NCHUNK):
                            nc.tensor.matmul(
                                pss[c],
                                xb[:, k, :],
                                qw[:, k, c * 512:(c + 1) * 512],
                                start=(k == 0 and h == 0),
                                stop=(k == ks - 1 and h == 1),
                            )
                for c in range(NCHUNK):
                    nc.scalar.copy(out=ot[:, c * 512:(c + 1) * 512],
                                   in_=pss[c])
                nc.scalar.dma_start(
                    out=y.ap()[t * 128:(t + 1) * 128, :], in_=ot,
                )
                if t + 1 < t_tiles:
                    xcur = xnext
    nc.compile()
    return nc


def kernel(x: np.ndarray, weight: np.ndarray) -> np.ndarray:
    global LAST_RESULTS
    assert x.shape == (B, S, D_IN) and weight.shape == (D_OUT, D_IN)

    thresh = _threshold(weight)

    # Host-side layout prep: transpose for partition-major DMA.
    xT = np.ascontiguousarray(x.reshape(T_TOK, D_IN).T.astype(np.float32,
                                                              copy=False))
    wT = np.ascontiguousarray(weight.T.astype(np.float32, copy=False))

    nc = build_program(thresh)
    in_maps = [
        {"xT": xT,
         "wT": np.ascontiguousarray(wT[:, c * O_PER:(c + 1) * O_PER])}
        for c in range(N_CORES)
    ]
    res = bass_utils.run_bass_kernel_spmd(
        nc, in_maps, list(range(N_CORES)), trace=TRACE, tmpdir=TMPDIR,
    )
    LAST_RESULTS = res
    y = np.concatenate([res.results[c]["y"] for c in range(N_CORES)], axis=1)
    return np.ascontiguousarray(y.reshape(B, S, D_OUT).astype(np.float32,
                                                              copy=False))



# revision 2
# speedup vs baseline: 1.6660x; 1.6660x over previous
"""BitLinear158 forward on 8 Trainium2 NeuronCores.

y = x @ quantize(W).T where quantize is the absmean ternary quantizer:
    gamma = mean(|W|) + 1e-6 ; qw = sign(W) * min(round(|W/gamma|), 1)

Strategy (tensor parallel over out_features, x replicated):
  - host: ternary-quantize W with the same jax path as the reference and
    cast to fp8 e4m3 ({-1,0,1} are exact). Split x into e4m3 hi + e4m3
    residual (PASSES=2) so the device sees only fp8 bytes.
  - each core: keep its W.T shard resident in SBUF, stream 128-token x
    tiles, and run fp8 DoubleRow matmuls (two 128-k slabs contracted per
    instruction at 2x fp8 rate) accumulating hi+lo passes in fp32 PSUM.
  - layouts are pre-tiled on host ([p, tile, kslab, tok]) so every DMA is
    a long contiguous per-partition burst.
"""

import numpy as np

import concourse.bass as bass
import concourse.bacc as bacc
import concourse.mybir as mybir
import concourse.tile as tile
from concourse import bass_utils

# Problem shapes (hardcoded per contract).
B, S, D_IN, D_OUT = 4, 2048, 4096, 16384
N_CORES = 8
O_PER = D_OUT // N_CORES          # 2048 out-features per core
T_TOK = B * S                     # 8192 tokens
KS = D_IN // 128                  # 32 k-slabs of 128
TT = T_TOK // 128                 # 64 token tiles
EPS = 1e-6

# 2 = fp8 hi + fp8 residual (rel err ~5e-4); 1 = single fp8 pass (~1.6e-2).
PASSES = 2

# Set by test harness to capture profiling info; leave False for grading.
TRACE = False
TMPDIR = None
LAST_RESULTS = None


def _quantize_ref(weight: np.ndarray) -> np.ndarray:
    """Bit-exact replication of reference.absmean_quantize (eager jax on the
    default backend, matching how the reference executes); numpy fallback."""
    try:
        import jax.numpy as jnp

        gamma = jnp.abs(weight).mean() + EPS
        ws = weight / gamma
        qw = jnp.sign(ws) * jnp.minimum(jnp.round(jnp.abs(ws)), 1.0)
        return np.asarray(qw)
    except Exception:
        gamma = np.float32(np.abs(weight).mean(dtype=np.float64)) + np.float32(EPS)
        ws = (weight / gamma).astype(np.float32)
        return (np.sign(ws) * np.minimum(np.round(np.abs(ws)), np.float32(1.0))
                ).astype(np.float32)


def build_program(t_tiles: int = TT, o_per: int = O_PER, ks: int = KS,
                  passes: int = PASSES) -> bass.Bass:
    """Emit the per-core Bass/Tile program.

    DRAM I/O (per core), all pre-tiled on host:
      xhi/xlo [128, t_tiles, ks, 128] e4m3  -- x passes, replicated
      w       [128, ks, o_per]        e4m3  -- this core's W.T shard
      y       [t_tiles, 128, o_per]   f32   -- this core's output slice
    """
    NCHUNK = o_per // 512
    npair = ks // 2
    fp8 = mybir.dt.float8e4

    nc = bacc.Bacc("TRN2", target_bir_lowering=False, debug=False)
    xdr = [nc.dram_tensor(nm, [128, t_tiles, ks, 128], fp8,
                          kind="ExternalInput")
           for nm in ("xhi", "xlo")[:passes]]
    wd = nc.dram_tensor("w", [128, ks, o_per], fp8, kind="ExternalInput")
    y = nc.dram_tensor("y", [t_tiles, 128, o_per], mybir.dt.float32,
                       kind="ExternalOutput")

    with tile.TileContext(nc) as tc:
        with (
            tc.tile_pool(name="wres", bufs=1) as w_pool,
            tc.tile_pool(name="xs", bufs=2) as x_pool,
            tc.tile_pool(name="outs", bufs=2) as out_pool,
            tc.tile_pool(name="psum", bufs=2, space="PSUM") as psum_pool,
        ):
            # ---- resident ternary weight shard, fp8 ----
            w8 = w_pool.tile([128, ks, o_per], fp8)
            nc.gpsimd.dma_start(out=w8, in_=wd.ap())

            def load_x(t):
                xs = []
                for i in range(passes):
                    xt = x_pool.tile([128, ks, 128], fp8,
                                     name=f"x{i}", tag=f"x{i}")
                    nc.gpsimd.dma_start(out=xt, in_=xdr[i].ap()[:, t, :, :])
                    xs.append(xt)
                return xs

            xcur = load_x(0)

            # ---- main loop over 128-token tiles ----
            for t in range(t_tiles):
                if t + 1 < t_tiles:
                    xnext = load_x(t + 1)

                pss = [psum_pool.tile([128, 512], mybir.dt.float32,
                                      name=f"ps{c}", tag=f"ps{c}")
                       for c in range(NCHUNK)]
                for p in range(passes):
                    for j in range(npair):
                        for c in range(NCHUNK):
                            nc.tensor.matmul(
                                pss[c],
                                xcur[p][:, 2 * j:2 * j + 2, :],
                                w8[:, 2 * j:2 * j + 2,
                                   c * 512:(c + 1) * 512],
                                start=(p == 0 and j == 0),
                                stop=(p == passes - 1 and j == npair - 1),
                                perf_mode=mybir.MatmulPerfMode.DoubleRow,
                            )
                ot = out_pool.tile([128, o_per], mybir.dt.float32)
                for c in range(NCHUNK):
                    nc.scalar.copy(out=ot[:, c * 512:(c + 1) * 512],
                                   in_=pss[c])
                nc.scalar.dma_start(out=y.ap()[t, :, :], in_=ot)
                if t + 1 < t_tiles:
                    xcur = xnext
    nc.compile()
    return nc


def _x_dev(a8: np.ndarray) -> np.ndarray:
    """[T_TOK, D_IN] fp8 -> [128p, TT, KS, 128tok] device layout."""
    return np.ascontiguousarray(
        a8.reshape(TT, 128, KS, 128).transpose(3, 0, 2, 1))


def kernel(x: np.ndarray, weight: np.ndarray) -> np.ndarray:
    global LAST_RESULTS
    import ml_dtypes
    fp8 = ml_dtypes.float8_e4m3
    assert x.shape == (B, S, D_IN) and weight.shape == (D_OUT, D_IN)

    qw = _quantize_ref(weight).astype(np.float32, copy=False)

    xf = np.ascontiguousarray(x.reshape(T_TOK, D_IN)).astype(np.float32,
                                                             copy=False)
    xhi8 = xf.astype(fp8)
    xs = [_x_dev(xhi8)]
    if PASSES == 2:
        xlo8 = (xf - xhi8.astype(np.float32)).astype(fp8)
        xs.append(_x_dev(xlo8))

    # W.T in device layout [128p, KS, D_OUT] fp8, then per-core o-slices.
    w8 = np.ascontiguousarray(
        qw.T.reshape(KS, 128, D_OUT).transpose(1, 0, 2)).astype(fp8)

    nc = build_program()
    in_maps = []
    for c in range(N_CORES):
        m = {nm: arr for nm, arr in zip(("xhi", "xlo"), xs)}
        m["w"] = np.ascontiguousarray(w8[:, :, c * O_PER:(c + 1) * O_PER])
        in_maps.append(m)
    res = bass_utils.run_bass_kernel_spmd(
        nc, in_maps, list(range(N_CORES)), trace=TRACE, tmpdir=TMPDIR,
    )
    LAST_RESULTS = res
    y = np.concatenate(
        [res.results[c]["y"].reshape(T_TOK, O_PER) for c in range(N_CORES)],
        axis=1)
    return np.ascontiguousarray(y.reshape(B, S, D_OUT).astype(np.float32,
                                                              copy=False))


# revision 7
# speedup vs baseline: 1.6664x; 1.0003x over previous
"""BitLinear158 forward on 8 Trainium2 NeuronCores.

y = x @ quantize(W).T where quantize is the absmean ternary quantizer:
    gamma = mean(|W|) + 1e-6 ; qw = sign(W) * min(round(|W/gamma|), 1)

Strategy (tensor parallel over out_features, x replicated):
  - host: ternary-quantize W with the same jax path as the reference and
    cast to fp8 e4m3 ({-1,0,1} are exact). Split x into e4m3 hi + e4m3
    residual (PASSES=2) so the device sees only fp8 bytes.
  - each core: keep its W.T shard resident in SBUF, stream 128-token x
    tiles, and run fp8 DoubleRow matmuls (two 128-k slabs contracted per
    instruction at 2x fp8 rate) accumulating hi+lo passes in fp32 PSUM.
  - layouts are pre-tiled on host ([p, tile, kslab, tok]) so every DMA is
    a long contiguous per-partition burst.
"""

import numpy as np

import concourse.bass as bass
import concourse.bacc as bacc
import concourse.mybir as mybir
import concourse.tile as tile
from concourse import bass_utils

# Problem shapes (hardcoded per contract).
B, S, D_IN, D_OUT = 4, 2048, 4096, 16384
N_CORES = 8
O_PER = D_OUT // N_CORES          # 2048 out-features per core
T_TOK = B * S                     # 8192 tokens
KS = D_IN // 128                  # 32 k-slabs of 128
TT = T_TOK // 128                 # 64 token tiles
EPS = 1e-6

# 2 = fp8 hi + fp8 residual (rel err ~5e-4); 1 = single fp8 pass (~1.6e-2).
PASSES = 2

# Set by test harness to capture profiling info; leave False for grading.
TRACE = False
TMPDIR = None
LAST_RESULTS = None


def _quantize_ref(weight: np.ndarray) -> np.ndarray:
    """Bit-exact replication of reference.absmean_quantize (eager jax on the
    default backend, matching how the reference executes); numpy fallback."""
    try:
        import jax.numpy as jnp

        gamma = jnp.abs(weight).mean() + EPS
        ws = weight / gamma
        qw = jnp.sign(ws) * jnp.minimum(jnp.round(jnp.abs(ws)), 1.0)
        return np.asarray(qw)
    except Exception:
        gamma = np.float32(np.abs(weight).mean(dtype=np.float64)) + np.float32(EPS)
        ws = (weight / gamma).astype(np.float32)
        return (np.sign(ws) * np.minimum(np.round(np.abs(ws)), np.float32(1.0))
                ).astype(np.float32)


def build_program(t_tiles: int = TT, o_per: int = O_PER, ks: int = KS,
                  passes: int = PASSES) -> bass.Bass:
    """Emit the per-core Bass/Tile program.

    DRAM I/O (per core), all pre-tiled on host:
      xhi/xlo [128, t_tiles, ks, 128] e4m3  -- x passes, replicated
      w       [128, ks, o_per]        e4m3  -- this core's W.T shard
      y       [t_tiles, 128, o_per]   f32   -- this core's output slice
    """
    NCHUNK = o_per // 512
    npair = ks // 2
    fp8 = mybir.dt.float8e4

    nc = bacc.Bacc("TRN2", target_bir_lowering=False, debug=False)
    xdr = [nc.dram_tensor(nm, [128, t_tiles, ks, 128], fp8,
                          kind="ExternalInput")
           for nm in ("xhi", "xlo")[:passes]]
    wd = nc.dram_tensor("w", [128, npair, o_per, 2], fp8,
                        kind="ExternalInput")
    y = nc.dram_tensor("y", [t_tiles, 128, o_per], mybir.dt.float32,
                       kind="ExternalOutput")

    with tile.TileContext(nc) as tc:
        with (
            tc.tile_pool(name="wres", bufs=1) as w_pool,
            tc.tile_pool(name="xs", bufs=2) as x_pool,
            tc.tile_pool(name="outs", bufs=2) as out_pool,
            tc.tile_pool(name="psum", bufs=2, space="PSUM") as psum_pool,
        ):
            # ---- resident ternary weight shard, fp8, pair-interleaved ----
            w8 = w_pool.tile([128, npair, o_per, 2], fp8)
            nc.gpsimd.dma_start(out=w8, in_=wd.ap())

            def load_x(t):
                xs = []
                for i in range(passes):
                    xt = x_pool.tile([128, ks, 128], fp8,
                                     name=f"x{i}", tag=f"x{i}")
                    nc.gpsimd.dma_start(out=xt, in_=xdr[i].ap()[:, t])
                    xs.append(xt)
                return xs

            xcur = load_x(0)

            # ---- main loop over 128-token tiles ----
            for t in range(t_tiles):
                if t + 1 < t_tiles:
                    xnext = load_x(t + 1)

                pss = [psum_pool.tile([128, 512], mybir.dt.float32,
                                      name=f"ps{c}", tag=f"ps{c}")
                       for c in range(NCHUNK)]
                for p in range(passes):
                    for j in range(npair):
                        for c in range(NCHUNK):
                            nc.tensor.matmul(
                                pss[c],
                                xcur[p][:, 2 * j:2 * j + 2, :],
                                w8[:, j, c * 512:(c + 1) * 512,
                                   :].transpose([0, 2, 1]),
                                start=(p == 0 and j == 0),
                                stop=(p == passes - 1 and j == npair - 1),
                                perf_mode=mybir.MatmulPerfMode.DoubleRow,
                            )
                ot = out_pool.tile([128, o_per], mybir.dt.float32)
                for c in range(NCHUNK):
                    nc.scalar.copy(out=ot[:, c * 512:(c + 1) * 512],
                                   in_=pss[c])
                nc.scalar.dma_start(out=y.ap()[t, :, :], in_=ot)
                if t + 1 < t_tiles:
                    xcur = xnext
    nc.compile()
    return nc


def _x_dev(a8: np.ndarray) -> np.ndarray:
    """[T_TOK, D_IN] fp8 -> [128p, TT, KS, 128tok] device layout."""
    return np.ascontiguousarray(
        a8.reshape(TT, 128, KS, 128).transpose(3, 0, 2, 1))


def kernel(x: np.ndarray, weight: np.ndarray) -> np.ndarray:
    global LAST_RESULTS
    import ml_dtypes
    fp8 = ml_dtypes.float8_e4m3
    assert x.shape == (B, S, D_IN) and weight.shape == (D_OUT, D_IN)

    qw = _quantize_ref(weight).astype(np.float32, copy=False)

    xf = np.ascontiguousarray(x.reshape(T_TOK, D_IN)).astype(np.float32,
                                                             copy=False)
    xhi8 = xf.astype(fp8)
    xs = [_x_dev(xhi8)]
    if PASSES == 2:
        xlo8 = (xf - xhi8.astype(np.float32)).astype(fp8)
        xs.append(_x_dev(xlo8))

    # W.T in device layout [128p, KS/2, D_OUT, 2] fp8 (pair-interleaved),
    # then per-core o-slices.
    w8 = np.ascontiguousarray(
        qw.T.reshape(KS // 2, 2, 128, D_OUT).transpose(2, 0, 3, 1)).astype(fp8)

    nc = build_program()
    in_maps = []
    for c in range(N_CORES):
        m = {nm: arr for nm, arr in zip(("xhi", "xlo"), xs)}
        m["w"] = np.ascontiguousarray(w8[:, :, c * O_PER:(c + 1) * O_PER, :])
        in_maps.append(m)
    res = bass_utils.run_bass_kernel_spmd(
        nc, in_maps, list(range(N_CORES)), trace=TRACE, tmpdir=TMPDIR,
    )
    LAST_RESULTS = res
    y = np.concatenate(
        [res.results[c]["y"].reshape(T_TOK, O_PER) for c in range(N_CORES)],
        axis=1)
    return np.ascontiguousarray(y.reshape(B, S, D_OUT).astype(np.float32,
                                                              copy=False))


# revision 8
# speedup vs baseline: 3.9028x; 2.3420x over previous
"""BitLinear158 forward on 8 Trainium2 NeuronCores.

y = x @ quantize(W).T where quantize is the absmean ternary quantizer:
    gamma = mean(|W|) + 1e-6 ; qw = sign(W) * min(round(|W/gamma|), 1)

Strategy (tensor parallel over out_features, x replicated):
  - host: ternary-quantize W with the same jax path as the reference and
    cast to fp8 e4m3 ({-1,0,1} are exact). Quantize x to e4m3 (PASSES=1)
    or e4m3 hi + e4m3 residual (PASSES=2) so the device sees only fp8.
  - each core: keep its W.T shard resident in SBUF, stream 128-token x
    tiles, and run fp8 DoubleRow matmuls (two 128-k slabs contracted per
    instruction at 2x fp8 rate) accumulating in fp32 PSUM.
  - layouts are pre-tiled on host so every DMA is a long contiguous
    per-partition burst; W is loaded per 512-out chunk so the PE can
    start as soon as the first chunk lands.
"""

import numpy as np

import concourse.bass as bass
import concourse.bacc as bacc
import concourse.mybir as mybir
import concourse.tile as tile
from concourse import bass_utils

# Problem shapes (hardcoded per contract).
B, S, D_IN, D_OUT = 4, 2048, 4096, 16384
N_CORES = 8
O_PER = D_OUT // N_CORES          # 2048 out-features per core
T_TOK = B * S                     # 8192 tokens
KS = D_IN // 128                  # 32 k-slabs of 128
TT = T_TOK // 128                 # 64 token tiles
EPS = 1e-6

# 1 = single fp8 pass (rel err ~1.6e-2); 2 = fp8 hi + residual (~5e-4).
PASSES = 1

# Set by test harness to capture profiling info; leave False for grading.
TRACE = False
TMPDIR = None
LAST_RESULTS = None


def _quantize_ref(weight: np.ndarray) -> np.ndarray:
    """Bit-exact replication of reference.absmean_quantize (eager jax on the
    default backend, matching how the reference executes); numpy fallback."""
    try:
        import jax.numpy as jnp

        gamma = jnp.abs(weight).mean() + EPS
        ws = weight / gamma
        qw = jnp.sign(ws) * jnp.minimum(jnp.round(jnp.abs(ws)), 1.0)
        return np.asarray(qw)
    except Exception:
        gamma = np.float32(np.abs(weight).mean(dtype=np.float64)) + np.float32(EPS)
        ws = (weight / gamma).astype(np.float32)
        return (np.sign(ws) * np.minimum(np.round(np.abs(ws)), np.float32(1.0))
                ).astype(np.float32)


def build_program(t_tiles: int = TT, o_per: int = O_PER, ks: int = KS,
                  passes: int = PASSES) -> bass.Bass:
    """Emit the per-core Bass/Tile program.

    DRAM I/O (per core), all pre-tiled on host:
      xhi/xlo [128, t_tiles, ks, 128] e4m3  -- x passes, replicated
      w       [128, ks, o_per]        e4m3  -- this core's W.T shard
      y       [t_tiles, 128, o_per]   f32   -- this core's output slice
    """
    NCHUNK = o_per // 512
    npair = ks // 2
    fp8 = mybir.dt.float8e4

    nc = bacc.Bacc("TRN2", target_bir_lowering=False, debug=False)
    xdr = [nc.dram_tensor(nm, [128, t_tiles, ks, 128], fp8,
                          kind="ExternalInput")
           for nm in ("xhi", "xlo")[:passes]]
    wd = nc.dram_tensor("w", [128, ks, o_per], fp8, kind="ExternalInput")
    y = nc.dram_tensor("y", [t_tiles, 128, o_per], mybir.dt.float32,
                       kind="ExternalOutput")

    with tile.TileContext(nc) as tc:
        with (
            tc.tile_pool(name="wres", bufs=1) as w_pool,
            tc.tile_pool(name="xs", bufs=3) as x_pool,
            tc.tile_pool(name="outs", bufs=2) as out_pool,
            tc.tile_pool(name="psum", bufs=2, space="PSUM") as psum_pool,
        ):
            def load_x(t):
                xs = []
                for i in range(passes):
                    xt = x_pool.tile([128, ks, 128], fp8,
                                     name=f"x{i}", tag=f"x{i}")
                    nc.gpsimd.dma_start(out=xt, in_=xdr[i].ap()[:, t])
                    xs.append(xt)
                return xs

            xq = [load_x(0)]

            # ---- resident ternary weight shard, fp8, loaded per chunk ----
            wcs = []
            for c in range(NCHUNK):
                wc = w_pool.tile([128, ks, 512], fp8, name=f"w{c}",
                                 tag=f"w{c}")
                nc.gpsimd.dma_start(
                    out=wc, in_=wd.ap()[:, :, c * 512:(c + 1) * 512])
                wcs.append(wc)

            xq.append(load_x(1))

            # ---- main loop over 128-token tiles ----
            for t in range(t_tiles):
                xcur = xq.pop(0)
                if t + 2 < t_tiles:
                    xq.append(load_x(t + 2))

                pss = [psum_pool.tile([128, 512], mybir.dt.float32,
                                      name=f"ps{c}", tag=f"ps{c}")
                       for c in range(NCHUNK)]
                for p in range(passes):
                    for j in range(npair):
                        for c in range(NCHUNK):
                            nc.tensor.matmul(
                                pss[c],
                                xcur[p][:, 2 * j:2 * j + 2, :],
                                wcs[c][:, 2 * j:2 * j + 2, :],
                                start=(p == 0 and j == 0),
                                stop=(p == passes - 1 and j == npair - 1),
                                perf_mode=mybir.MatmulPerfMode.DoubleRow,
                            )
                ot = out_pool.tile([128, o_per], mybir.dt.float32)
                for c in range(NCHUNK):
                    nc.scalar.copy(out=ot[:, c * 512:(c + 1) * 512],
                                   in_=pss[c])
                    nc.scalar.dma_start(
                        out=y.ap()[t, :, c * 512:(c + 1) * 512],
                        in_=ot[:, c * 512:(c + 1) * 512])
    nc.compile()
    return nc


def _x_dev(a8: np.ndarray) -> np.ndarray:
    """[T_TOK, D_IN] fp8 -> [128p, TT, KS, 128tok] device layout."""
    return np.ascontiguousarray(
        a8.reshape(TT, 128, KS, 128).transpose(3, 0, 2, 1))


def kernel(x: np.ndarray, weight: np.ndarray) -> np.ndarray:
    global LAST_RESULTS
    import ml_dtypes
    fp8 = ml_dtypes.float8_e4m3
    assert x.shape == (B, S, D_IN) and weight.shape == (D_OUT, D_IN)

    qw = _quantize_ref(weight).astype(np.float32, copy=False)

    xf = np.ascontiguousarray(x.reshape(T_TOK, D_IN)).astype(np.float32,
                                                             copy=False)
    xhi8 = xf.astype(fp8)
    xs = [_x_dev(xhi8)]
    if PASSES == 2:
        xlo8 = (xf - xhi8.astype(np.float32)).astype(fp8)
        xs.append(_x_dev(xlo8))

    # W.T in device layout [128p, KS, D_OUT] fp8, then per-core o-slices.
    w8 = np.ascontiguousarray(
        qw.T.reshape(KS, 128, D_OUT).transpose(1, 0, 2)).astype(fp8)

    nc = build_program()
    in_maps = []
    for c in range(N_CORES):
        m = {nm: arr for nm, arr in zip(("xhi", "xlo"), xs)}
        m["w"] = np.ascontiguousarray(w8[:, :, c * O_PER:(c + 1) * O_PER])
        in_maps.append(m)
    res = bass_utils.run_bass_kernel_spmd(
        nc, in_maps, list(range(N_CORES)), trace=TRACE, tmpdir=TMPDIR,
    )
    LAST_RESULTS = res
    y = np.concatenate(
        [res.results[c]["y"].reshape(T_TOK, O_PER) for c in range(N_CORES)],
        axis=1)
    return np.ascontiguousarray(y.reshape(B, S, D_OUT).astype(np.float32,
                                                              copy=False))
